# revision 52
# baseline (speedup 1.0000x reference)
"""Trainium2 Bass kernel for a dense transformer block (B=2, T=2048, C=1024, 16 heads).

Sharding: data-parallel over batch (2 groups of 4 cores) x tensor-parallel
within each group (4 heads + 1024 MLP hidden per core). The T=2048 rows are
processed in SIX non-uniform chunks [256, 256, 512, 512, 384, 128]: small
chunks at the head fill the collective pipeline sooner, and the 128-row tail
chunk shrinks the un-overlappable final op->AR->LN2->MLP->RS chain from
~130us to ~40us.

  LN1+QKV(chunk) -> attention(chunk) -> out-proj -> AllReduce(bf16)
    -> residual+LN2 (replicated in group) -> MLP -> ReduceScatter(bf16) -> out

Other perf features (see round-1 notes): ln/exp-shared ACT tables for the LN
rstd, engine rebalance (residual adds on GpSimd, epilogue evacuations on
Vector, b_o/TP folded into the out-proj evacuation), deferred attention
epilogue closures interleaved between matmuls, and head DMA reordering.
"""
import functools
import os
import sys
import types

sys.path.insert(0, "/opt/trn_rl_repo")

import numpy as np
import ml_dtypes

import concourse.bass as bass
import concourse.mybir as mybir
from concourse import tile
import concourse.bass_utils as bass_utils

BF16 = ml_dtypes.bfloat16
F32 = np.float32
dt = mybir.dt
AF = mybir.ActivationFunctionType
ALU = mybir.AluOpType

B, T, C = 2, 2048, 1024
NH, HS = 16, 64
NCORES = 8
TP = 4                      # tensor-parallel group size
GROUPS = [[0, 1, 2, 3], [4, 5, 6, 7]]
HPR = NH // TP              # heads per rank
CHR = HPR * HS              # attn channels per rank (256)
HIDR = 4 * C // TP          # MLP hidden per rank (1024)
RPC = T // TP               # rows per core (512)
EPS = 1e-5
NCT = C // 128              # C tiles (8)
NRT = T // 128              # row tiles over full T (16)

CHUNKS = [(0, 256), (256, 256), (512, 512), (1024, 512), (1536, 384), (1920, 128)]
NCH = len(CHUNKS)


# ---------------------------------------------------------------------------
# Harness fixups: the walrus in this container caps sync-wait commands per
# instruction, but Tile's kernel-tail drain carries one wait per active
# processor. Split those waits onto individual SP nops ahead of the drain.
def _patched_drain_and_barrier(self, tick_clock, wait_clock):
    nc = self.nc
    probe = mybir.InstNoOp(
        name=nc.get_next_instruction_name(),
        engine=mybir.EngineType.SP,
        bass_nofuse=True,
    )
    wait_clock.add_sem_waits(probe, tile.ScopedClock({None: tick_clock.global_clock}))
    waits = list(probe.sync_info.on_wait) if probe.sync_info is not None else []
    for w in waits:
        nop = nc.sync.nop(nofuse=True, hint="split_tail_wait")
        nop.ins.sync_info = mybir.SyncInfo(on_wait=[w], on_update=[])
    nc.sync.drain()
    nc.all_engine_barrier()
    assert self.sems is not None
    popped = nc._tile_sem_poison_stack.pop()
    assert popped is self._sem_poison
    nc.clear_and_free_semaphores(list(self.sems.allocated().values()))
    nc.all_engine_barrier()


tile.TileContext._drain_and_barrier = _patched_drain_and_barrier


def _install_ntff_hook():
    """antenv.axon_hooks is absent from this image; provide it and register
    the ctypes NTFF profile hook so trace=True yields exec_time_ns."""
    if "antenv.axon_hooks" in sys.modules:
        return
    import antenv

    mod = types.ModuleType("antenv.axon_hooks")
    mod._hook = None
    mod.set_axon_ntff_profile_hook = lambda h: setattr(mod, "_hook", h)
    mod.get_axon_ntff_profile_hook = lambda: mod._hook
    sys.modules["antenv.axon_hooks"] = mod
    antenv.axon_hooks = mod
    try:
        from trn_agent_boot.trn_boot import _ntff_profile_via_ctypes

        hook = _ntff_profile_via_ctypes("/opt/axon/libaxon_pjrt.so")
        if hook is not None:
            mod.set_axon_ntff_profile_hook(hook)
    except Exception:
        pass
    bass_utils.upload_artifacts = lambda tmpdir: f"local://{tmpdir}"

    import concourse.bass2jax as b2j

    orig_hook = b2j.neuronx_cc_hook

    def dbg_hook(*a, **k):
        try:
            return orig_hook(*a, **k)
        except BaseException:
            import traceback

            traceback.print_exc()
            raise

    b2j.neuronx_cc_hook = dbg_hook


_install_ntff_hook()


_SYNC_WAIT_LIMIT = 1


def _split_sync_waits(nc, limit=_SYNC_WAIT_LIMIT):
    """Walrus in this container rejects instructions with more than a couple
    of sync-wait commands; hoist excess waits onto same-engine NOPs placed
    immediately before the offending instruction."""
    n_split = 0
    for fn in nc.m.functions:
        for bb in fn.blocks:
            new_insts = []
            for inst in bb.instructions:
                si = inst.sync_info
                if si is not None and si.on_wait is not None and len(si.on_wait) > limit:
                    waits = list(si.on_wait)
                    for idx, w in enumerate(waits[limit:]):
                        nop = mybir.InstNoOp(
                            name=f"{inst.name}-sw{idx}",
                            engine=inst.engine,
                            bass_nofuse=True,
                            sync_info=mybir.SyncInfo(on_wait=[w], on_update=[]),
                        )
                        new_insts.append(nop)
                        n_split += 1
                    inst.sync_info = mybir.SyncInfo(
                        on_wait=waits[:limit], on_update=list(si.on_update)
                    )
                new_insts.append(inst)
            bb.instructions = new_insts
    return n_split


# ---------------------------------------------------------------------------
def _build_nc() -> bass.Bass:
    nc = bass.Bass("TRN2", num_devices=NCORES, num_swdge_queues=4)

    x_b = nc.dram_tensor("x_b", [T, C], dt.float32, kind="ExternalInput")
    wq = nc.dram_tensor("wq", [C, CHR], dt.bfloat16, kind="ExternalInput")
    wk = nc.dram_tensor("wk", [C, CHR], dt.bfloat16, kind="ExternalInput")
    wv = nc.dram_tensor("wv", [C, CHR], dt.bfloat16, kind="ExternalInput")
    bq = nc.dram_tensor("bq", [128, 2], dt.float32, kind="ExternalInput")
    bk = nc.dram_tensor("bk", [128, 2], dt.float32, kind="ExternalInput")
    bvb = nc.dram_tensor("bvb", [128, CHR], dt.float32, kind="ExternalInput")
    wo = nc.dram_tensor("wo", [CHR, C], dt.bfloat16, kind="ExternalInput")
    bob = nc.dram_tensor("bob", [128, C], dt.float32, kind="ExternalInput")
    w1 = nc.dram_tensor("w1", [C, HIDR], dt.bfloat16, kind="ExternalInput")
    b1 = nc.dram_tensor("b1", [128, HIDR // 128], dt.float32, kind="ExternalInput")
    w2 = nc.dram_tensor("w2", [HIDR, C], dt.bfloat16, kind="ExternalInput")
    bq4 = nc.dram_tensor("bq4", [128, C], dt.float32, kind="ExternalInput")
    ident = nc.dram_tensor("ident", [128, 128], dt.bfloat16, kind="ExternalInput")
    maskut = nc.dram_tensor("maskut", [128, 128], dt.bfloat16, kind="ExternalInput")
    out = nc.dram_tensor("out", [RPC, C], dt.bfloat16, kind="ExternalOutput")

    with tile.TileContext(nc) as tc:
        with (
            tc.tile_pool(name="dram", bufs=1, space="DRAM") as dram,
            tc.tile_pool(name="const", bufs=1) as cpool,
            tc.tile_pool(name="hT", bufs=1) as hTpool,
            tc.tile_pool(name="kqv", bufs=1) as kqvpool,
            tc.tile_pool(name="att", bufs=1) as attpool,
        ):
            # attn-out partials cross the AllReduce in fp8e4m3: partials are
            # O(0.1) and 4-way summed, so quantization lands ~2e-3 in the
            # final output while halving the dominant collective's traffic
            rs1_in = [dram.tile([r, C], dt.float8e4, name=f"rs1i{c}", tag=f"rs1i{c}")
                      for c, (s, r) in enumerate(CHUNKS)]
            ar1_out = [dram.tile([r, C], dt.float8e4, name=f"ar1o{c}", tag=f"ar1o{c}")
                       for c, (s, r) in enumerate(CHUNKS)]
            rs2_in = [dram.tile([r, C], dt.bfloat16, name=f"rs2i{c}", tag=f"rs2i{c}")
                      for c, (s, r) in enumerate(CHUNKS)]
            rs2_out = [dram.tile([r // 4, C], dt.bfloat16, name=f"rs2o{c}", tag=f"rs2o{c}")
                       for c, (s, r) in enumerate(CHUNKS)]
            warm_in = dram.tile([128, 4], dt.float32, name="warm_i", tag="warm_i")
            warm_out = dram.tile([TP * 128, 4], dt.float32, name="warm_o", tag="warm_o")
            nc.gpsimd.collective_compute(
                "AllGather", ALU.bypass, replica_groups=GROUPS,
                ins=[warm_in[:].opt()], outs=[warm_out[:].opt()],
            )

            # ---- early consts on the sync queue (first transposes need id)
            id_sb = cpool.tile([128, 128], dt.bfloat16, name="id", tag="id")
            nc.sync.dma_start(id_sb[:], ident[:])
            mask_sb = cpool.tile([128, 128], dt.bfloat16, name="mask", tag="mask")
            nc.sync.dma_start(mask_sb[:], maskut[:])
            eps_sb = cpool.tile([128, 1], dt.float32, name="eps", tag="eps")
            nc.vector.memset(eps_sb[:], EPS)

            # ---- QKV-phase weights on the scalar queue (needed first)
            wq_sb = cpool.tile([128, NCT, CHR], dt.bfloat16, name="wq", tag="wq")
            wk_sb = cpool.tile([128, NCT, CHR], dt.bfloat16, name="wk", tag="wk")
            wv_sb = cpool.tile([128, NCT, CHR], dt.bfloat16, name="wv", tag="wv")
            nc.scalar.dma_start(wk_sb[:], wk.rearrange("(j p) o -> p j o", p=128))
            nc.scalar.dma_start(wq_sb[:], wq.rearrange("(j p) o -> p j o", p=128))
            nc.scalar.dma_start(wv_sb[:], wv.rearrange("(j p) o -> p j o", p=128))
            bq_sb = cpool.tile([128, 2], dt.float32, name="bq", tag="bq")
            bk_sb = cpool.tile([128, 2], dt.float32, name="bk", tag="bk")
            nc.scalar.dma_start(bq_sb[:], bq[:])
            nc.scalar.dma_start(bk_sb[:], bk[:])
            bvb_sb = cpool.tile([128, CHR], dt.float32, name="bvb", tag="bvb")
            nc.scalar.dma_start(bvb_sb[:], bvb[:])

            # ---- later-phase weights ride the gpsimd queue
            wo_sb = cpool.tile([128, 2, C], dt.bfloat16, name="wo", tag="wo")
            nc.gpsimd.dma_start(wo_sb[:], wo.rearrange("(t p) o -> p t o", p=128))
            w1_sb = cpool.tile([128, NCT, HIDR], dt.bfloat16, name="w1", tag="w1")
            nc.gpsimd.dma_start(w1_sb[:], w1.rearrange("(j p) o -> p j o", p=128))
            w2_sb = cpool.tile([128, HIDR // 128, C], dt.bfloat16, name="w2", tag="w2")
            nc.gpsimd.dma_start(w2_sb[:], w2.rearrange("(j p) o -> p j o", p=128))
            bob_sb = cpool.tile([128, C], dt.float32, name="bob", tag="bob")
            nc.gpsimd.dma_start(bob_sb[:], bob[:])
            b1_sb = cpool.tile([128, HIDR // 128], dt.float32, name="b1", tag="b1")
            nc.gpsimd.dma_start(b1_sb[:], b1[:])
            bq4_sb = cpool.tile([128, C], dt.float32, name="bq4", tag="bq4")
            nc.gpsimd.dma_start(bq4_sb[:], bq4[:])

            hT = hTpool.tile([128, NCT, T], dt.bfloat16, name="hT", tag="hT")
            kt = [kqvpool.tile([128, T], dt.bfloat16, name=f"kt{h2}", tag=f"kt{h2}") for h2 in range(2)]
            qt = [kqvpool.tile([128, T], dt.bfloat16, name=f"qt{h2}", tag=f"qt{h2}") for h2 in range(2)]
            vaug = kqvpool.tile([128, NRT, HPR, HS + 1], dt.bfloat16, name="vaug", tag="vaug")
            aT = [attpool.tile([128, T], dt.bfloat16, name=f"aT{h2}", tag=f"aT{h2}") for h2 in range(2)]
            h2T = [attpool.tile([128, NCT, r], dt.bfloat16, name=f"h2T{c}", tag=f"h2T{c % 2}")
                   for c, (s, r) in enumerate(CHUNKS)]
            zb_tiles = {}

            with (
                tc.tile_pool(name="lnx", bufs=2) as lxpool,
                tc.tile_pool(name="lnsp", bufs=6) as spool,
                tc.tile_pool(name="lnh", bufs=6) as hpool,
                tc.tile_pool(name="zt", bufs=2) as zpool,
                tc.tile_pool(name="zb", bufs=4) as zbpool,
                tc.tile_pool(name="pt", bufs=6) as ptpool,
                tc.tile_pool(name="anat", bufs=4) as anpool,
                tc.tile_pool(name="small", bufs=6) as smpool,
                tc.tile_pool(name="ob", bufs=3) as obpool,
                tc.tile_pool(name="ut", bufs=8) as utpool,
                tc.tile_pool(name="mb", bufs=2) as mbpool,
                tc.tile_pool(name="psb", bufs=6, space="PSUM") as psb,
                tc.tile_pool(name="psa", bufs=2, space="PSUM") as psapool,
            ):
                # LN of a [128, C] f32 tile -> bf16 (gain/bias folded downstream).
                # rstd = exp(-0.5*ln(var+eps)): ln/exp live in one activation
                # table set, so this never forces an ACT table reload (Sqrt did).
                def ln_tile(src_ap, dst_ap):
                    st6 = spool.tile([128, 2, 6], dt.float32, name="st6", tag="st6")
                    nc.vector.bn_stats(st6[:, 0, :], src_ap[:, 0:512])
                    nc.vector.bn_stats(st6[:, 1, :], src_ap[:, 512:1024])
                    st2 = spool.tile([128, 2], dt.float32, name="st2", tag="st2")
                    nc.vector.bn_aggr(st2[:], st6[:])
                    lnv = spool.tile([128, 1], dt.float32, name="lnv", tag="lnv")
                    nc.scalar.activation(lnv[:], st2[:, 1:2], AF.Ln, bias=eps_sb[:])
                    rstd = spool.tile([128, 1], dt.float32, name="rstd", tag="rstd")
                    nc.scalar.activation(rstd[:], lnv[:], AF.Exp, scale=-0.5)
                    nc.vector.tensor_scalar(
                        dst_ap, src_ap, st2[:, 0:1], rstd[:],
                        op0=ALU.subtract, op1=ALU.mult,
                    )

                def transpose_128(dst_ap, src_ap):
                    pst = psb.tile([128, 128], dt.bfloat16, name="pst", tag="psb")
                    nc.tensor.transpose(pst[:], src_ap, id_sb[:])
                    nc.scalar.copy(dst_ap, pst[:])

                def transpose_pair(dst_ap, src_ap):
                    # two transposes share one PSUM tile so a single strided
                    # ACT copy evacuates both -- halves the copy instruction
                    # count on the exp-contended scalar engine
                    pst = psb.tile([128, 2, 128], dt.bfloat16, name="pst", tag="psb")
                    nc.tensor.transpose(pst[:, 0, :], src_ap[:, 0:128], id_sb[:])
                    nc.tensor.transpose(pst[:, 1, :], src_ap[:, 128:256], id_sb[:])
                    nc.scalar.copy(dst_ap, pst[:])

                def lnqkv_steps(cc):
                    """LN1 + transposes + QKV/V for chunk cc (generator)."""
                    start, rows = CHUNKS[cc]
                    tiles = rows // 128
                    t0 = start // 128

                    def load(tl):
                        i = t0 + tl
                        xt = lxpool.tile([128, C], dt.float32, name="xt", tag="xt")
                        nc.sync.dma_start(xt[:], x_b[i * 128:(i + 1) * 128, :])
                        return xt

                    nxt_xt = load(0)
                    for tl in range(tiles):
                        i = t0 + tl
                        xt = nxt_xt
                        if tl < tiles - 1:
                            nxt_xt = load(tl + 1)
                        h = hpool.tile([128, C], dt.bfloat16, name="h", tag="h")
                        ln_tile(xt[:], h[:])
                        for jp in range(NCT // 2):
                            transpose_pair(hT[:, 2 * jp:2 * jp + 2, i * 128:(i + 1) * 128],
                                           h[:, jp * 256:(jp + 1) * 256])
                        yield
                    for h2 in range(2):
                        for w_sb, t_sb, b_sb in ((wk_sb, kt, bk_sb), (wq_sb, qt, bq_sb)):
                            ps = psb.tile([128, rows], dt.float32, name="psqk", tag="psb")
                            for j in range(NCT):
                                nc.tensor.matmul(
                                    ps[:],
                                    w_sb[:, j, h2 * 128:(h2 + 1) * 128],
                                    hT[:, j, start:start + rows],
                                    start=(j == 0), stop=(j == NCT - 1),
                                )
                            nc.scalar.activation(
                                t_sb[h2][:, start:start + rows], ps[:],
                                AF.Identity, bias=b_sb[:, h2:h2 + 1],
                            )
                            yield
                    for tl in range(tiles):
                        i = t0 + tl
                        ps = psb.tile([128, CHR], dt.float32, name="psv", tag="psb")
                        for j in range(NCT):
                            nc.tensor.matmul(
                                ps[:],
                                hT[:, j, i * 128:(i + 1) * 128],
                                wv_sb[:, j, :],
                                start=(j == 0), stop=(j == NCT - 1),
                            )
                        nc.vector.tensor_tensor(
                            vaug[:, i, :, 0:HS],
                            ps[:].rearrange("p (h d) -> p h d", d=HS),
                            bvb_sb[:].rearrange("p (h d) -> p h d", d=HS),
                            op=ALU.add,
                        )
                        nc.vector.memset(vaug[:, i, :, HS:HS + 1], 1.0)
                        yield

                def attn_steps(rc):
                    start, rows = CHUNKS[rc]
                    tiles = rows // 128
                    t0 = start // 128
                    kmax = t0 + tiles - 1
                    epi_q = []

                    def drain_epi(n):
                        for _ in range(n):
                            if epi_q:
                                epi_q.pop(0)()

                    for h2 in range(2):
                        psATs = [
                            psapool.tile([HS + 1, rows], dt.float32, name=f"psAT{sub}", tag="psa")
                            for sub in range(2)
                        ]

                        def scores_step(ki, h2=h2):
                            rel = max(0, ki * 128 - start)
                            pts = []
                            for sub in range(2):
                                pb = sub * 64
                                psS = psb.tile([128, rows], dt.float32, name="psS", tag="psb")
                                nc.tensor.matmul(
                                    psS[:, rel:rows],
                                    kt[h2][pb:pb + 64, ki * 128:(ki + 1) * 128],
                                    qt[h2][pb:pb + 64, start + rel:start + rows],
                                    start=True, stop=True,
                                )
                                pt = ptpool.tile([128, rows], dt.bfloat16, name="pt", tag="pt")
                                nc.scalar.activation(pt[:, rel:rows], psS[:, rel:rows], AF.Exp)
                                if rel > 0:
                                    nc.vector.memset(pt[:, 0:rel], 0.0)
                                if ki * 128 - start >= 0:
                                    nc.vector.tensor_tensor(
                                        pt[:, rel:rel + 128], pt[:, rel:rel + 128],
                                        mask_sb[:], op=ALU.mult,
                                    )
                                pts.append(pt)
                            return pts

                        pending = scores_step(0)
                        for ki in range(kmax + 1):
                            nxt = scores_step(ki + 1) if ki < kmax else None
                            for sub in range(2):
                                nc.tensor.matmul(
                                    psATs[sub][:],
                                    vaug[:, ki, h2 * 2 + sub, :],
                                    pending[sub][:],
                                    start=(ki == 0), stop=(ki == kmax),
                                )
                            drain_epi(2)
                            pending = nxt
                            yield

                        # evacuate AV psum now (frees psa for the next h2);
                        # the per-tile normalize/transpose chains are deferred
                        # and drained between later matmuls.
                        avts = []
                        for sub in range(2):
                            avt = anpool.tile([HS + 1, rows], dt.bfloat16, name="avt", tag="avt")
                            nc.vector.tensor_copy(avt[:], psATs[sub][:])
                            avts.append(avt)

                        prevB = None
                        for sub in range(2):
                            pb = sub * 64
                            avt = avts[sub]
                            for tl in range(tiles):
                                t_abs = t0 + tl
                                hold = {}

                                def stepA(avt=avt, tl=tl, hold=hold):
                                    psN = psb.tile([128, HS + 1], dt.bfloat16, name="psN", tag="psb")
                                    nc.tensor.transpose(
                                        psN[:], avt[:, tl * 128:(tl + 1) * 128],
                                        id_sb[0:HS + 1, 0:HS + 1],
                                    )
                                    rden = smpool.tile([128, 1], dt.float32, name="rden", tag="rden")
                                    nc.vector.reciprocal(rden[:], psN[:, HS:HS + 1])
                                    anat = anpool.tile([128, HS], dt.bfloat16, name="anat", tag="anat")
                                    nc.vector.tensor_scalar(
                                        anat[:], psN[:, 0:HS], rden[:], None, op0=ALU.mult
                                    )
                                    hold['anat'] = anat

                                def stepB(hold=hold, pb=pb, h2=h2, t_abs=t_abs):
                                    anat = hold['anat']
                                    psT2 = psb.tile([64, 128], dt.bfloat16, name="psT2", tag="psb")
                                    nc.tensor.transpose(psT2[:], anat[:], id_sb[:])
                                    nc.vector.tensor_copy(
                                        aT[h2][pb:pb + 64, t_abs * 128:(t_abs + 1) * 128],
                                        psT2[:],
                                    )

                                epi_q.append(stepA)
                                if prevB is not None:
                                    epi_q.append(prevB)
                                prevB = stepB
                        if prevB is not None:
                            epi_q.append(prevB)

                    while epi_q:
                        drain_epi(2)
                        yield

                def outproj_compute(rc):
                    # split from the AR trigger: the matmuls are emitted ahead
                    # of the PREVIOUS chunk's ln2 so the PE chews out-proj
                    # while the DVE runs that ln2's residual/stats chain; the
                    # trigger stays after ln2's at-loads, keeping the gpsimd
                    # and CC instruction order unchanged.
                    start, rows = CHUNKS[rc]
                    for tl in range(rows // 128):
                        i_abs = start // 128 + tl
                        ob = obpool.tile([128, C], dt.float8e4, name="ob", tag="ob")
                        for nh in range(2):
                            psO = psb.tile([128, 512], dt.float32, name="psO", tag="psb")
                            for ct in range(2):
                                nc.tensor.matmul(
                                    psO[:],
                                    aT[ct][:, i_abs * 128:(i_abs + 1) * 128],
                                    wo_sb[:, ct, nh * 512:(nh + 1) * 512],
                                    start=(ct == 0), stop=(ct == 1),
                                )
                            # fold b_o/TP into the evacuation (summed to b_o by AR)
                            nc.vector.tensor_tensor(
                                ob[:, nh * 512:(nh + 1) * 512], psO[:],
                                bob_sb[:, nh * 512:(nh + 1) * 512], op=ALU.add,
                            )
                        nc.sync.dma_start(rs1_in[rc][tl * 128:(tl + 1) * 128, :], ob[:])

                def outproj_trigger(rc):
                    nc.gpsimd.collective_compute(
                        "AllReduce", ALU.add, replica_groups=GROUPS,
                        ins=[rs1_in[rc][:].opt()], outs=[ar1_out[rc][:].opt()],
                    )

                def outproj_chunk(rc):
                    outproj_compute(rc)
                    outproj_trigger(rc)

                def ln2_chunk(rc):
                    # replicated: all rows of the chunk on every rank
                    start, rows = CHUNKS[rc]
                    tiles = rows // 128

                    def load(tl):
                        at = obpool.tile([128, C], dt.float8e4, name="at", tag="ob")
                        nc.gpsimd.dma_start(at[:], ar1_out[rc][tl * 128:(tl + 1) * 128, :])
                        xt = lxpool.tile([128, C], dt.float32, name="xt2", tag="xt")
                        i = start // 128 + tl
                        nc.sync.dma_start(xt[:], x_b[i * 128:(i + 1) * 128, :])
                        return at, xt

                    # fp8 'at' is read on DVE (critical path); the off-path
                    # zb staging rides the pool except for the exposed tail
                    zb_eng = nc.vector if rc >= 4 else nc.gpsimd
                    pre = load(0)
                    h2ns = []
                    for tl in range(tiles):
                        at, xt = pre
                        if tl + 1 < tiles:
                            pre = load(tl + 1)
                        z = zpool.tile([128, C], dt.float32, name="z", tag="z")
                        nc.vector.tensor_tensor(z[:], at[:], xt[:], op=ALU.add)
                        h2n = hpool.tile([128, C], dt.bfloat16, name="h2n", tag="h")
                        ln_tile(z[:], h2n[:])
                        h2ns.append(h2n)
                        # z + b2 staged in bf16; the mb evacuation folds the
                        # /TP (RS then sums to z + b2 + ff)
                        zb = zbpool.tile([128, C], dt.bfloat16, name="zbt", tag="zbt")
                        zb_eng.tensor_tensor(zb[:], z[:], bq4_sb[:], op=ALU.add)
                        zb_tiles[(rc, tl)] = zb
                    # transposes batched after the DVE chains: the PE streams
                    # them in one dense burst instead of stalling per tile
                    for tl in range(tiles):
                        for jp in range(NCT // 2):
                            transpose_pair(h2T[rc][:, 2 * jp:2 * jp + 2, tl * 128:(tl + 1) * 128],
                                           h2ns[tl][:, jp * 256:(jp + 1) * 256])

                def mlp_steps(rc):
                    start, rows = CHUNKS[rc]
                    tiles = rows // 128
                    uts = []
                    for ht in range(HIDR // 128):
                        psU = psb.tile([128, rows], dt.float32, name="psU", tag="psb")
                        for j in range(NCT):
                            nc.tensor.matmul(
                                psU[:],
                                w1_sb[:, j, ht * 128:(ht + 1) * 128],
                                h2T[rc][:, j, :],
                                start=(j == 0), stop=(j == NCT - 1),
                            )
                        ut = utpool.tile([128, rows], dt.bfloat16, name="ut", tag="ut")
                        nc.vector.tensor_copy(ut[:], psU[:])
                        uts.append(ut)
                        yield
                    # gelu in place: ut holds u then gelu(u + b1)
                    for ht in range(HIDR // 128):
                        nc.scalar.activation(
                            uts[ht][:], uts[ht][:], AF.Gelu, bias=b1_sb[:, ht:ht + 1]
                        )
                    gts = uts
                    yield
                    for tl in range(tiles):
                        mb = mbpool.tile([128, C], dt.bfloat16, name="mb", tag="mb")
                        for nh in range(2):
                            psD = psb.tile([128, 512], dt.float32, name="psD", tag="psb")
                            for ht in range(HIDR // 128):
                                nc.tensor.matmul(
                                    psD[:],
                                    gts[ht][:, tl * 128:(tl + 1) * 128],
                                    w2_sb[:, ht, nh * 512:(nh + 1) * 512],
                                    start=(ht == 0), stop=(ht == HIDR // 128 - 1),
                                )
                            # mb = (z + b2)/TP + psD; RS sums to z + b2 + ff
                            nc.vector.scalar_tensor_tensor(
                                mb[:, nh * 512:(nh + 1) * 512],
                                zb_tiles[(rc, tl)][:, nh * 512:(nh + 1) * 512],
                                1.0 / TP, psD[:], op0=ALU.mult, op1=ALU.add,
                            )
                        nc.sync.dma_start(rs2_in[rc][tl * 128:(tl + 1) * 128, :], mb[:])
                        yield
                    nc.gpsimd.collective_compute(
                        "ReduceScatter", ALU.add, replica_groups=GROUPS,
                        ins=[rs2_in[rc][:].opt()], outs=[rs2_out[rc][:].opt()],
                    )
                    lo = start // 4
                    nc.gpsimd.dma_start(out[lo:lo + rows // 4, :], rs2_out[rc][:])

                def drain(gen):
                    for _ in gen:
                        pass

                def interleave(gen_a, gen_b, na, nb):
                    """Merge two instruction generators proportionally."""
                    ia = ib = 0
                    done_a = done_b = False
                    while not (done_a and done_b):
                        pick_a = (not done_a) and (done_b or ia * nb <= ib * na)
                        if pick_a:
                            try:
                                next(gen_a)
                                ia += 1
                            except StopIteration:
                                done_a = True
                        else:
                            try:
                                next(gen_b)
                                ib += 1
                            except StopIteration:
                                done_b = True

                def n_attn(rc):
                    start, rows = CHUNKS[rc]
                    return 2 * ((start + rows) // 128) + 4 * (rows // 128)

                def n_lnqkv(cc):
                    return 2 * (CHUNKS[cc][1] // 128) + 4

                def n_mlp(rc):
                    return 8 + 1 + CHUNKS[rc][1] // 128

                # ---- interleaved chunk-pipelined schedule
                drain(lnqkv_steps(0))
                interleave(attn_steps(0), lnqkv_steps(1), n_attn(0), n_lnqkv(1))
                outproj_chunk(0)                     # AR1(0)
                interleave(attn_steps(1), lnqkv_steps(2), n_attn(1), n_lnqkv(2))
                outproj_compute(1)
                ln2_chunk(0)
                outproj_trigger(1)                   # AR1(1)
                interleave(attn_steps(2), lnqkv_steps(3), n_attn(2), n_lnqkv(3))
                drain(mlp_steps(0))                  # RS2(0)
                outproj_compute(2)
                ln2_chunk(1)
                outproj_trigger(2)                   # AR1(2)
                interleave(attn_steps(3), lnqkv_steps(4), n_attn(3), n_lnqkv(4))
                drain(mlp_steps(1))                  # RS2(1)
                outproj_compute(3)
                ln2_chunk(2)
                outproj_trigger(3)                   # AR1(3)
                interleave(attn_steps(4), lnqkv_steps(5), n_attn(4), n_lnqkv(5))
                drain(mlp_steps(2))                  # RS2(2)
                outproj_compute(4)
                ln2_chunk(3)
                outproj_trigger(4)                   # AR1(4)
                interleave(attn_steps(5), mlp_steps(3), n_attn(5), n_mlp(3))  # RS2(3)
                outproj_compute(5)
                ln2_chunk(4)
                outproj_trigger(5)                   # AR1(5)
                drain(mlp_steps(4))                  # RS2(4)
                ln2_chunk(5)
                drain(mlp_steps(5))                  # RS2(5)

    _split_sync_waits(nc)
    return nc


@functools.lru_cache(maxsize=1)
def _get_nc():
    return _build_nc()


def _make_in_maps(inputs):
    x = np.asarray(inputs["x"], F32)
    W_qkv = np.asarray(inputs["W_qkv"], F32)
    b_qkv = np.asarray(inputs["b_qkv"], F32)
    W_o = np.asarray(inputs["W_o"], F32)
    b_o = np.asarray(inputs["b_o"], F32)
    ln1_g = np.asarray(inputs["ln1_g"], F32)
    ln1_b = np.asarray(inputs["ln1_b"], F32)
    ln2_g = np.asarray(inputs["ln2_g"], F32)
    ln2_b = np.asarray(inputs["ln2_b"], F32)
    W1 = np.asarray(inputs["W1"], F32)
    b1 = np.asarray(inputs["b1"], F32)
    W2 = np.asarray(inputs["W2"], F32)
    b2 = np.asarray(inputs["b2"], F32)

    scale = HS ** -0.5
    Wqkv_f = ln1_g[:, None] * W_qkv
    bqkv_f = ln1_b @ W_qkv + b_qkv
    Kw, Qw, Vw = Wqkv_f[:, :C], Wqkv_f[:, C:2 * C], Wqkv_f[:, 2 * C:]
    bK, bQ, bV = bqkv_f[:C], bqkv_f[C:2 * C], bqkv_f[2 * C:]
    W1f = ln2_g[:, None] * W1
    b1f = ln2_b @ W1 + b1

    ident = np.eye(128, dtype=BF16)
    mask = np.triu(np.ones((128, 128), dtype=F32)).astype(BF16)
    # b_o/TP folded into each rank's out-proj evacuation (AR sums to b_o)
    bob = np.ascontiguousarray(np.broadcast_to(b_o / TP, (128, C))).astype(F32)
    b2qc = np.ascontiguousarray(np.broadcast_to(b2, (128, C))).astype(F32)

    in_maps = []
    for core in range(NCORES):
        g, r = divmod(core, TP)
        hs = slice(CHR * r, CHR * (r + 1))
        hid = slice(HIDR * r, HIDR * (r + 1))
        xg = x[g]
        m = {
            "x_b": np.ascontiguousarray(xg),
            "wq": np.ascontiguousarray(Qw[:, hs] * scale).astype(BF16),
            "wk": np.ascontiguousarray(Kw[:, hs]).astype(BF16),
            "wv": np.ascontiguousarray(Vw[:, hs]).astype(BF16),
            "bq": np.ascontiguousarray((bQ[hs] * scale).reshape(2, 128).T),
            "bk": np.ascontiguousarray(bK[hs].reshape(2, 128).T),
            "bvb": np.ascontiguousarray(np.broadcast_to(bV[hs], (128, CHR))),
            "wo": np.ascontiguousarray(W_o[hs, :]).astype(BF16),
            "bob": bob,
            "w1": np.ascontiguousarray(W1f[:, hid]).astype(BF16),
            "b1": np.ascontiguousarray(b1f[hid].reshape(HIDR // 128, 128).T),
            "w2": np.ascontiguousarray(W2[hid, :]).astype(BF16),
            "bq4": b2qc,
            "ident": ident,
            "maskut": mask,
        }
        in_maps.append(m)
    return in_maps


def _run(inputs, trace=False):
    nc = _get_nc()
    in_maps = _make_in_maps(inputs)
    res = bass_utils.run_bass_kernel_spmd(
        nc, in_maps, core_ids=list(range(NCORES)), trace=trace
    )
    out = np.empty((B, T, C), F32)
    for core in range(NCORES):
        g, r = divmod(core, TP)
        o = np.asarray(res.results[core]["out"], dtype=F32)
        for (start, rows) in CHUNKS:
            q = rows // 4
            lo = start // 4
            out[g, start + r * q: start + (r + 1) * q] = o[lo:lo + q]
    return out, res


def kernel(**inputs) -> np.ndarray:
    out, _ = _run(inputs, trace=False)
    return out


# revision 59
# speedup vs baseline: 1.1684x; 1.1684x over previous
"""Trainium2 Bass kernel for a dense transformer block (B=2, T=2048, C=1024, 16 heads).

Sharding: data-parallel over batch (2 groups of 4 cores) x tensor-parallel
within each group (4 heads + 1024 MLP hidden per core). The T=2048 rows are
processed in SIX non-uniform chunks [256, 256, 512, 512, 384, 128]: small
chunks at the head fill the collective pipeline sooner, and the 128-row tail
chunk shrinks the un-overlappable final op->AR->LN2->MLP->RS chain from
~130us to ~40us.

  LN1+QKV(chunk) -> attention(chunk) -> out-proj -> AllReduce(bf16)
    -> residual+LN2 (replicated in group) -> MLP -> ReduceScatter(bf16) -> out

Other perf features (see round-1 notes): ln/exp-shared ACT tables for the LN
rstd, engine rebalance (residual adds on GpSimd, epilogue evacuations on
Vector, b_o/TP folded into the out-proj evacuation), deferred attention
epilogue closures interleaved between matmuls, and head DMA reordering.
"""
import functools
import os
import sys
import types

sys.path.insert(0, "/opt/trn_rl_repo")

import numpy as np
import ml_dtypes

import concourse.bass as bass
import concourse.mybir as mybir
from concourse import tile
import concourse.bass_utils as bass_utils

BF16 = ml_dtypes.bfloat16
F32 = np.float32
dt = mybir.dt
AF = mybir.ActivationFunctionType
ALU = mybir.AluOpType

B, T, C = 2, 2048, 1024
NH, HS = 16, 64
NCORES = 8
TP = 4                      # tensor-parallel group size
GROUPS = [[0, 1, 2, 3], [4, 5, 6, 7]]
HPR = NH // TP              # heads per rank
CHR = HPR * HS              # attn channels per rank (256)
HIDR = 4 * C // TP          # MLP hidden per rank (1024)
RPC = T // TP               # rows per core (512)
EPS = 1e-5
NCT = C // 128              # C tiles (8)
NRT = T // 128              # row tiles over full T (16)

CHUNKS = [(0, 256), (256, 256), (512, 512), (1024, 512), (1536, 384), (1920, 128)]
NCH = len(CHUNKS)


# ---------------------------------------------------------------------------
# Harness fixups: the walrus in this container caps sync-wait commands per
# instruction, but Tile's kernel-tail drain carries one wait per active
# processor. Split those waits onto individual SP nops ahead of the drain.
def _patched_drain_and_barrier(self, tick_clock, wait_clock):
    nc = self.nc
    probe = mybir.InstNoOp(
        name=nc.get_next_instruction_name(),
        engine=mybir.EngineType.SP,
        bass_nofuse=True,
    )
    wait_clock.add_sem_waits(probe, tile.ScopedClock({None: tick_clock.global_clock}))
    waits = list(probe.sync_info.on_wait) if probe.sync_info is not None else []
    for w in waits:
        nop = nc.sync.nop(nofuse=True, hint="split_tail_wait")
        nop.ins.sync_info = mybir.SyncInfo(on_wait=[w], on_update=[])
    nc.sync.drain()
    nc.all_engine_barrier()
    assert self.sems is not None
    popped = nc._tile_sem_poison_stack.pop()
    assert popped is self._sem_poison
    nc.clear_and_free_semaphores(list(self.sems.allocated().values()))
    nc.all_engine_barrier()


tile.TileContext._drain_and_barrier = _patched_drain_and_barrier


def _install_ntff_hook():
    """antenv.axon_hooks is absent from this image; provide it and register
    the ctypes NTFF profile hook so trace=True yields exec_time_ns."""
    if "antenv.axon_hooks" in sys.modules:
        return
    import antenv

    mod = types.ModuleType("antenv.axon_hooks")
    mod._hook = None
    mod.set_axon_ntff_profile_hook = lambda h: setattr(mod, "_hook", h)
    mod.get_axon_ntff_profile_hook = lambda: mod._hook
    sys.modules["antenv.axon_hooks"] = mod
    antenv.axon_hooks = mod
    try:
        from trn_agent_boot.trn_boot import _ntff_profile_via_ctypes

        hook = _ntff_profile_via_ctypes("/opt/axon/libaxon_pjrt.so")
        if hook is not None:
            mod.set_axon_ntff_profile_hook(hook)
    except Exception:
        pass
    bass_utils.upload_artifacts = lambda tmpdir: f"local://{tmpdir}"

    import concourse.bass2jax as b2j

    orig_hook = b2j.neuronx_cc_hook

    def dbg_hook(*a, **k):
        try:
            return orig_hook(*a, **k)
        except BaseException:
            import traceback

            traceback.print_exc()
            raise

    b2j.neuronx_cc_hook = dbg_hook


_install_ntff_hook()


_SYNC_WAIT_LIMIT = 1


def _split_sync_waits(nc, limit=_SYNC_WAIT_LIMIT):
    """Walrus in this container rejects instructions with more than a couple
    of sync-wait commands; hoist excess waits onto same-engine NOPs placed
    immediately before the offending instruction."""
    n_split = 0
    for fn in nc.m.functions:
        for bb in fn.blocks:
            new_insts = []
            for inst in bb.instructions:
                si = inst.sync_info
                if si is not None and si.on_wait is not None and len(si.on_wait) > limit:
                    waits = list(si.on_wait)
                    for idx, w in enumerate(waits[limit:]):
                        nop = mybir.InstNoOp(
                            name=f"{inst.name}-sw{idx}",
                            engine=inst.engine,
                            bass_nofuse=True,
                            sync_info=mybir.SyncInfo(on_wait=[w], on_update=[]),
                        )
                        new_insts.append(nop)
                        n_split += 1
                    inst.sync_info = mybir.SyncInfo(
                        on_wait=waits[:limit], on_update=list(si.on_update)
                    )
                new_insts.append(inst)
            bb.instructions = new_insts
    return n_split


# ---------------------------------------------------------------------------
def _build_nc() -> bass.Bass:
    nc = bass.Bass("TRN2", num_devices=NCORES, num_swdge_queues=4)

    x_b = nc.dram_tensor("x_b", [T, C], dt.float32, kind="ExternalInput")
    wq = nc.dram_tensor("wq", [C, CHR], dt.bfloat16, kind="ExternalInput")
    wk = nc.dram_tensor("wk", [C, CHR], dt.bfloat16, kind="ExternalInput")
    wv = nc.dram_tensor("wv", [C, CHR], dt.bfloat16, kind="ExternalInput")
    bq = nc.dram_tensor("bq", [128, 2], dt.float32, kind="ExternalInput")
    bk = nc.dram_tensor("bk", [128, 2], dt.float32, kind="ExternalInput")
    bvb = nc.dram_tensor("bvb", [128, CHR], dt.float32, kind="ExternalInput")
    wo = nc.dram_tensor("wo", [CHR, C], dt.bfloat16, kind="ExternalInput")
    bob = nc.dram_tensor("bob", [128, C], dt.float32, kind="ExternalInput")
    w1 = nc.dram_tensor("w1", [C, HIDR], dt.bfloat16, kind="ExternalInput")
    b1 = nc.dram_tensor("b1", [128, HIDR // 128], dt.float32, kind="ExternalInput")
    w2 = nc.dram_tensor("w2", [HIDR, C], dt.bfloat16, kind="ExternalInput")
    bq4 = nc.dram_tensor("bq4", [128, C], dt.float32, kind="ExternalInput")
    ident = nc.dram_tensor("ident", [128, 128], dt.bfloat16, kind="ExternalInput")
    maskut = nc.dram_tensor("maskut", [128, 128], dt.bfloat16, kind="ExternalInput")
    out = nc.dram_tensor("out", [RPC, C], dt.bfloat16, kind="ExternalOutput")

    with tile.TileContext(nc) as tc:
        with (
            tc.tile_pool(name="dram", bufs=1, space="DRAM") as dram,
            tc.tile_pool(name="const", bufs=1) as cpool,
            tc.tile_pool(name="hT", bufs=1) as hTpool,
            tc.tile_pool(name="kqv", bufs=1) as kqvpool,
            tc.tile_pool(name="att", bufs=1) as attpool,
        ):
            # attn-out partials cross the AllReduce in fp8e4m3: partials are
            # O(0.1) and 4-way summed, so quantization lands ~2e-3 in the
            # final output while halving the dominant collective's traffic
            rs1_in = [dram.tile([r, C], dt.float8e4, name=f"rs1i{c}", tag=f"rs1i{c}")
                      for c, (s, r) in enumerate(CHUNKS)]
            ar1_out = [dram.tile([r, C], dt.float8e4, name=f"ar1o{c}", tag=f"ar1o{c}")
                       for c, (s, r) in enumerate(CHUNKS)]
            rs2_in = [dram.tile([r, C], dt.bfloat16, name=f"rs2i{c}", tag=f"rs2i{c}")
                      for c, (s, r) in enumerate(CHUNKS)]
            rs2_out = [dram.tile([r // 4, C], dt.bfloat16, name=f"rs2o{c}", tag=f"rs2o{c}")
                       for c, (s, r) in enumerate(CHUNKS)]
            warm_in = dram.tile([128, 4], dt.float32, name="warm_i", tag="warm_i")
            warm_out = dram.tile([TP * 128, 4], dt.float32, name="warm_o", tag="warm_o")
            nc.gpsimd.collective_compute(
                "AllGather", ALU.bypass, replica_groups=GROUPS,
                ins=[warm_in[:].opt()], outs=[warm_out[:].opt()],
            )

            # ---- early consts on the sync queue (first transposes need id)
            id_sb = cpool.tile([128, 128], dt.bfloat16, name="id", tag="id")
            nc.sync.dma_start(id_sb[:], ident[:])
            mask_sb = cpool.tile([128, 128], dt.bfloat16, name="mask", tag="mask")
            nc.sync.dma_start(mask_sb[:], maskut[:])
            eps_sb = cpool.tile([128, 1], dt.float32, name="eps", tag="eps")
            nc.vector.memset(eps_sb[:], EPS)

            # ---- QKV-phase weights on the scalar queue (needed first)
            wq_sb = cpool.tile([128, NCT, CHR], dt.bfloat16, name="wq", tag="wq")
            wk_sb = cpool.tile([128, NCT, CHR], dt.bfloat16, name="wk", tag="wk")
            wv_sb = cpool.tile([128, NCT, CHR], dt.bfloat16, name="wv", tag="wv")
            nc.scalar.dma_start(wk_sb[:], wk.rearrange("(j p) o -> p j o", p=128))
            nc.scalar.dma_start(wq_sb[:], wq.rearrange("(j p) o -> p j o", p=128))
            nc.scalar.dma_start(wv_sb[:], wv.rearrange("(j p) o -> p j o", p=128))
            bq_sb = cpool.tile([128, 2], dt.float32, name="bq", tag="bq")
            bk_sb = cpool.tile([128, 2], dt.float32, name="bk", tag="bk")
            nc.scalar.dma_start(bq_sb[:], bq[:])
            nc.scalar.dma_start(bk_sb[:], bk[:])
            bvb_sb = cpool.tile([128, CHR], dt.float32, name="bvb", tag="bvb")
            nc.scalar.dma_start(bvb_sb[:], bvb[:])

            # ---- later-phase weights ride the gpsimd queue
            wo_sb = cpool.tile([128, 2, C], dt.bfloat16, name="wo", tag="wo")
            nc.gpsimd.dma_start(wo_sb[:], wo.rearrange("(t p) o -> p t o", p=128))
            w1_sb = cpool.tile([128, NCT, HIDR], dt.bfloat16, name="w1", tag="w1")
            nc.gpsimd.dma_start(w1_sb[:], w1.rearrange("(j p) o -> p j o", p=128))
            w2_sb = cpool.tile([128, HIDR // 128, C], dt.bfloat16, name="w2", tag="w2")
            nc.gpsimd.dma_start(w2_sb[:], w2.rearrange("(j p) o -> p j o", p=128))
            bob_sb = cpool.tile([128, C], dt.float32, name="bob", tag="bob")
            nc.gpsimd.dma_start(bob_sb[:], bob[:])
            b1_sb = cpool.tile([128, HIDR // 128], dt.float32, name="b1", tag="b1")
            nc.gpsimd.dma_start(b1_sb[:], b1[:])
            bq4_sb = cpool.tile([128, C], dt.float32, name="bq4", tag="bq4")
            nc.gpsimd.dma_start(bq4_sb[:], bq4[:])

            hT = hTpool.tile([128, NCT, T], dt.bfloat16, name="hT", tag="hT")
            kt = [kqvpool.tile([128, T], dt.bfloat16, name=f"kt{h2}", tag=f"kt{h2}") for h2 in range(2)]
            qt = [kqvpool.tile([128, T], dt.bfloat16, name=f"qt{h2}", tag=f"qt{h2}") for h2 in range(2)]
            vaug = kqvpool.tile([128, NRT, HPR, HS + 1], dt.bfloat16, name="vaug", tag="vaug")
            aT = [attpool.tile([128, T], dt.bfloat16, name=f"aT{h2}", tag=f"aT{h2}") for h2 in range(2)]
            h2T = [attpool.tile([128, NCT, r], dt.bfloat16, name=f"h2T{c}", tag=f"h2T{c % 2}")
                   for c, (s, r) in enumerate(CHUNKS)]
            zb_tiles = {}

            with (
                tc.tile_pool(name="lnx", bufs=2) as lxpool,
                tc.tile_pool(name="lnsp", bufs=6) as spool,
                tc.tile_pool(name="lnh", bufs=6) as hpool,
                tc.tile_pool(name="zt", bufs=2) as zpool,
                tc.tile_pool(name="zb", bufs=4) as zbpool,
                tc.tile_pool(name="pt", bufs=6) as ptpool,
                tc.tile_pool(name="anat", bufs=4) as anpool,
                tc.tile_pool(name="small", bufs=6) as smpool,
                tc.tile_pool(name="ob", bufs=3) as obpool,
                tc.tile_pool(name="ut", bufs=8) as utpool,
                tc.tile_pool(name="mb", bufs=2) as mbpool,
                tc.tile_pool(name="psb", bufs=6, space="PSUM") as psb,
                tc.tile_pool(name="psa", bufs=2, space="PSUM") as psapool,
            ):
                # LN of a [128, C] f32 tile -> bf16 (gain/bias folded downstream).
                # rstd = exp(-0.5*ln(var+eps)): ln/exp live in one activation
                # table set, so this never forces an ACT table reload (Sqrt did).
                def ln_tile(src_ap, dst_ap):
                    st6 = spool.tile([128, 2, 6], dt.float32, name="st6", tag="st6")
                    nc.vector.bn_stats(st6[:, 0, :], src_ap[:, 0:512])
                    nc.vector.bn_stats(st6[:, 1, :], src_ap[:, 512:1024])
                    st2 = spool.tile([128, 2], dt.float32, name="st2", tag="st2")
                    nc.vector.bn_aggr(st2[:], st6[:])
                    lnv = spool.tile([128, 1], dt.float32, name="lnv", tag="lnv")
                    nc.scalar.activation(lnv[:], st2[:, 1:2], AF.Ln, bias=eps_sb[:])
                    rstd = spool.tile([128, 1], dt.float32, name="rstd", tag="rstd")
                    nc.scalar.activation(rstd[:], lnv[:], AF.Exp, scale=-0.5)
                    nc.vector.tensor_scalar(
                        dst_ap, src_ap, st2[:, 0:1], rstd[:],
                        op0=ALU.subtract, op1=ALU.mult,
                    )

                def transpose_128(dst_ap, src_ap):
                    pst = psb.tile([128, 128], dt.bfloat16, name="pst", tag="psb")
                    nc.tensor.transpose(pst[:], src_ap, id_sb[:])
                    nc.scalar.copy(dst_ap, pst[:])

                def transpose_pair(dst_ap, src_ap):
                    # two transposes share one PSUM tile so a single strided
                    # ACT copy evacuates both -- halves the copy instruction
                    # count on the exp-contended scalar engine
                    pst = psb.tile([128, 2, 128], dt.bfloat16, name="pst", tag="psb")
                    nc.tensor.transpose(pst[:, 0, :], src_ap[:, 0:128], id_sb[:])
                    nc.tensor.transpose(pst[:, 1, :], src_ap[:, 128:256], id_sb[:])
                    nc.scalar.copy(dst_ap, pst[:])

                def lnqkv_steps(cc):
                    """LN1 + transposes + QKV/V for chunk cc (generator)."""
                    start, rows = CHUNKS[cc]
                    tiles = rows // 128
                    t0 = start // 128

                    def load(tl):
                        i = t0 + tl
                        xt = lxpool.tile([128, C], dt.float32, name="xt", tag="xt")
                        nc.sync.dma_start(xt[:], x_b[i * 128:(i + 1) * 128, :])
                        return xt

                    nxt_xt = load(0)
                    for tl in range(tiles):
                        i = t0 + tl
                        xt = nxt_xt
                        if tl < tiles - 1:
                            nxt_xt = load(tl + 1)
                        h = hpool.tile([128, C], dt.bfloat16, name="h", tag="h")
                        ln_tile(xt[:], h[:])
                        for jp in range(NCT // 2):
                            transpose_pair(hT[:, 2 * jp:2 * jp + 2, i * 128:(i + 1) * 128],
                                           h[:, jp * 256:(jp + 1) * 256])
                        yield
                    for h2 in range(2):
                        for w_sb, t_sb, b_sb in ((wk_sb, kt, bk_sb), (wq_sb, qt, bq_sb)):
                            ps = psb.tile([128, rows], dt.float32, name="psqk", tag="psb")
                            for j in range(NCT):
                                nc.tensor.matmul(
                                    ps[:],
                                    w_sb[:, j, h2 * 128:(h2 + 1) * 128],
                                    hT[:, j, start:start + rows],
                                    start=(j == 0), stop=(j == NCT - 1),
                                )
                            nc.scalar.activation(
                                t_sb[h2][:, start:start + rows], ps[:],
                                AF.Identity, bias=b_sb[:, h2:h2 + 1],
                            )
                            yield
                    for tl in range(tiles):
                        i = t0 + tl
                        ps = psb.tile([128, CHR], dt.float32, name="psv", tag="psb")
                        for j in range(NCT):
                            nc.tensor.matmul(
                                ps[:],
                                hT[:, j, i * 128:(i + 1) * 128],
                                wv_sb[:, j, :],
                                start=(j == 0), stop=(j == NCT - 1),
                            )
                        nc.vector.tensor_tensor(
                            vaug[:, i, :, 0:HS],
                            ps[:].rearrange("p (h d) -> p h d", d=HS),
                            bvb_sb[:].rearrange("p (h d) -> p h d", d=HS),
                            op=ALU.add,
                        )
                        nc.vector.memset(vaug[:, i, :, HS:HS + 1], 1.0)
                        yield

                def attn_steps(rc):
                    start, rows = CHUNKS[rc]
                    tiles = rows // 128
                    t0 = start // 128
                    kmax = t0 + tiles - 1
                    epi_q = []

                    def drain_epi(n):
                        for _ in range(n):
                            if epi_q:
                                epi_q.pop(0)()

                    for h2 in range(2):
                        psATs = [
                            psapool.tile([HS + 1, rows], dt.float32, name=f"psAT{sub}", tag="psa")
                            for sub in range(2)
                        ]

                        def scores_step(ki, h2=h2):
                            rel = max(0, ki * 128 - start)
                            pts = []
                            for sub in range(2):
                                pb = sub * 64
                                psS = psb.tile([128, rows], dt.float32, name="psS", tag="psb")
                                nc.tensor.matmul(
                                    psS[:, rel:rows],
                                    kt[h2][pb:pb + 64, ki * 128:(ki + 1) * 128],
                                    qt[h2][pb:pb + 64, start + rel:start + rows],
                                    start=True, stop=True,
                                )
                                pt = ptpool.tile([128, rows], dt.bfloat16, name="pt", tag="pt")
                                nc.scalar.activation(pt[:, rel:rows], psS[:, rel:rows], AF.Exp)
                                if rel > 0:
                                    nc.vector.memset(pt[:, 0:rel], 0.0)
                                if ki * 128 - start >= 0:
                                    nc.vector.tensor_tensor(
                                        pt[:, rel:rel + 128], pt[:, rel:rel + 128],
                                        mask_sb[:], op=ALU.mult,
                                    )
                                pts.append(pt)
                            return pts

                        pending = scores_step(0)
                        for ki in range(kmax + 1):
                            nxt = scores_step(ki + 1) if ki < kmax else None
                            for sub in range(2):
                                nc.tensor.matmul(
                                    psATs[sub][:],
                                    vaug[:, ki, h2 * 2 + sub, :],
                                    pending[sub][:],
                                    start=(ki == 0), stop=(ki == kmax),
                                )
                            drain_epi(2)
                            pending = nxt
                            yield

                        # evacuate AV psum now (frees psa for the next h2);
                        # the per-tile normalize/transpose chains are deferred
                        # and drained between later matmuls.
                        avts = []
                        for sub in range(2):
                            avt = anpool.tile([HS + 1, rows], dt.bfloat16, name="avt", tag="avt")
                            nc.vector.tensor_copy(avt[:], psATs[sub][:])
                            avts.append(avt)

                        prevB = None
                        for sub in range(2):
                            pb = sub * 64
                            avt = avts[sub]
                            for tl in range(tiles):
                                t_abs = t0 + tl
                                hold = {}

                                def stepA(avt=avt, tl=tl, hold=hold):
                                    psN = psb.tile([128, HS + 1], dt.bfloat16, name="psN", tag="psb")
                                    nc.tensor.transpose(
                                        psN[:], avt[:, tl * 128:(tl + 1) * 128],
                                        id_sb[0:HS + 1, 0:HS + 1],
                                    )
                                    rden = smpool.tile([128, 1], dt.float32, name="rden", tag="rden")
                                    nc.vector.reciprocal(rden[:], psN[:, HS:HS + 1])
                                    anat = anpool.tile([128, HS], dt.bfloat16, name="anat", tag="anat")
                                    nc.vector.tensor_scalar(
                                        anat[:], psN[:, 0:HS], rden[:], None, op0=ALU.mult
                                    )
                                    hold['anat'] = anat

                                def stepB(hold=hold, pb=pb, h2=h2, t_abs=t_abs):
                                    anat = hold['anat']
                                    psT2 = psb.tile([64, 128], dt.bfloat16, name="psT2", tag="psb")
                                    nc.tensor.transpose(psT2[:], anat[:], id_sb[:])
                                    nc.vector.tensor_copy(
                                        aT[h2][pb:pb + 64, t_abs * 128:(t_abs + 1) * 128],
                                        psT2[:],
                                    )

                                epi_q.append(stepA)
                                if prevB is not None:
                                    epi_q.append(prevB)
                                prevB = stepB
                        if prevB is not None:
                            epi_q.append(prevB)

                    while epi_q:
                        drain_epi(2)
                        yield

                def outproj_chunk(rc):
                    start, rows = CHUNKS[rc]
                    for tl in range(rows // 128):
                        i_abs = start // 128 + tl
                        ob = obpool.tile([128, C], dt.float8e4, name="ob", tag="ob")
                        for nh in range(2):
                            psO = psb.tile([128, 512], dt.float32, name="psO", tag="psb")
                            for ct in range(2):
                                nc.tensor.matmul(
                                    psO[:],
                                    aT[ct][:, i_abs * 128:(i_abs + 1) * 128],
                                    wo_sb[:, ct, nh * 512:(nh + 1) * 512],
                                    start=(ct == 0), stop=(ct == 1),
                                )
                            # fold b_o/TP into the evacuation (summed to b_o by AR)
                            nc.vector.tensor_tensor(
                                ob[:, nh * 512:(nh + 1) * 512], psO[:],
                                bob_sb[:, nh * 512:(nh + 1) * 512], op=ALU.add,
                            )
                        nc.sync.dma_start(rs1_in[rc][tl * 128:(tl + 1) * 128, :], ob[:])
                    nc.gpsimd.collective_compute(
                        "AllReduce", ALU.add, replica_groups=GROUPS,
                        ins=[rs1_in[rc][:].opt()], outs=[ar1_out[rc][:].opt()],
                    )

                def ln2_chunk(rc):
                    # replicated: all rows of the chunk on every rank
                    start, rows = CHUNKS[rc]
                    tiles = rows // 128

                    def load(tl):
                        at = obpool.tile([128, C], dt.float8e4, name="at", tag="ob")
                        nc.gpsimd.dma_start(at[:], ar1_out[rc][tl * 128:(tl + 1) * 128, :])
                        xt = lxpool.tile([128, C], dt.float32, name="xt2", tag="xt")
                        i = start // 128 + tl
                        nc.sync.dma_start(xt[:], x_b[i * 128:(i + 1) * 128, :])
                        return at, xt

                    # fp8 'at' is read on DVE (critical path); the off-path
                    # zb staging rides the pool except for the exposed tail
                    zb_eng = nc.vector if rc >= 4 else nc.gpsimd
                    pre = load(0)
                    h2ns = []
                    for tl in range(tiles):
                        at, xt = pre
                        if tl + 1 < tiles:
                            pre = load(tl + 1)
                        z = zpool.tile([128, C], dt.float32, name="z", tag="z")
                        nc.vector.tensor_tensor(z[:], at[:], xt[:], op=ALU.add)
                        h2n = hpool.tile([128, C], dt.bfloat16, name="h2n", tag="h")
                        ln_tile(z[:], h2n[:])
                        h2ns.append(h2n)
                        # z + b2 staged in bf16; the mb evacuation folds the
                        # /TP (RS then sums to z + b2 + ff)
                        zb = zbpool.tile([128, C], dt.bfloat16, name="zbt", tag="zbt")
                        zb_eng.tensor_tensor(zb[:], z[:], bq4_sb[:], op=ALU.add)
                        zb_tiles[(rc, tl)] = zb
                    # transposes batched after the DVE chains: the PE streams
                    # them in one dense burst instead of stalling per tile
                    for tl in range(tiles):
                        for jp in range(NCT // 2):
                            transpose_pair(h2T[rc][:, 2 * jp:2 * jp + 2, tl * 128:(tl + 1) * 128],
                                           h2ns[tl][:, jp * 256:(jp + 1) * 256])

                def mlp_steps(rc):
                    start, rows = CHUNKS[rc]
                    tiles = rows // 128
                    uts = []
                    for ht in range(HIDR // 128):
                        psU = psb.tile([128, rows], dt.float32, name="psU", tag="psb")
                        for j in range(NCT):
                            nc.tensor.matmul(
                                psU[:],
                                w1_sb[:, j, ht * 128:(ht + 1) * 128],
                                h2T[rc][:, j, :],
                                start=(j == 0), stop=(j == NCT - 1),
                            )
                        ut = utpool.tile([128, rows], dt.bfloat16, name="ut", tag="ut")
                        nc.vector.tensor_copy(ut[:], psU[:])
                        uts.append(ut)
                        yield
                    # gelu in place: ut holds u then gelu(u + b1)
                    for ht in range(HIDR // 128):
                        nc.scalar.activation(
                            uts[ht][:], uts[ht][:], AF.Gelu, bias=b1_sb[:, ht:ht + 1]
                        )
                    gts = uts
                    yield
                    for tl in range(tiles):
                        mb = mbpool.tile([128, C], dt.bfloat16, name="mb", tag="mb")
                        for nh in range(2):
                            psD = psb.tile([128, 512], dt.float32, name="psD", tag="psb")
                            for ht in range(HIDR // 128):
                                nc.tensor.matmul(
                                    psD[:],
                                    gts[ht][:, tl * 128:(tl + 1) * 128],
                                    w2_sb[:, ht, nh * 512:(nh + 1) * 512],
                                    start=(ht == 0), stop=(ht == HIDR // 128 - 1),
                                )
                            # mb = (z + b2)/TP + psD; RS sums to z + b2 + ff
                            nc.vector.scalar_tensor_tensor(
                                mb[:, nh * 512:(nh + 1) * 512],
                                zb_tiles[(rc, tl)][:, nh * 512:(nh + 1) * 512],
                                1.0 / TP, psD[:], op0=ALU.mult, op1=ALU.add,
                            )
                        nc.sync.dma_start(rs2_in[rc][tl * 128:(tl + 1) * 128, :], mb[:])
                        yield
                    nc.gpsimd.collective_compute(
                        "ReduceScatter", ALU.add, replica_groups=GROUPS,
                        ins=[rs2_in[rc][:].opt()], outs=[rs2_out[rc][:].opt()],
                    )
                    lo = start // 4
                    nc.gpsimd.dma_start(out[lo:lo + rows // 4, :], rs2_out[rc][:])

                def drain(gen):
                    for _ in gen:
                        pass

                def interleave(gen_a, gen_b, na, nb):
                    """Merge two instruction generators proportionally."""
                    ia = ib = 0
                    done_a = done_b = False
                    while not (done_a and done_b):
                        pick_a = (not done_a) and (done_b or ia * nb <= ib * na)
                        if pick_a:
                            try:
                                next(gen_a)
                                ia += 1
                            except StopIteration:
                                done_a = True
                        else:
                            try:
                                next(gen_b)
                                ib += 1
                            except StopIteration:
                                done_b = True

                def n_attn(rc):
                    start, rows = CHUNKS[rc]
                    return 2 * ((start + rows) // 128) + 4 * (rows // 128)

                def n_lnqkv(cc):
                    return 2 * (CHUNKS[cc][1] // 128) + 4

                def n_mlp(rc):
                    return 8 + 1 + CHUNKS[rc][1] // 128

                # ---- interleaved chunk-pipelined schedule
                drain(lnqkv_steps(0))
                interleave(attn_steps(0), lnqkv_steps(1), n_attn(0), n_lnqkv(1))
                outproj_chunk(0)                     # AR1(0)
                interleave(attn_steps(1), lnqkv_steps(2), n_attn(1), n_lnqkv(2))
                ln2_chunk(0)
                outproj_chunk(1)                     # AR1(1)
                interleave(attn_steps(2), lnqkv_steps(3), n_attn(2), n_lnqkv(3))
                drain(mlp_steps(0))                  # RS2(0)
                ln2_chunk(1)
                outproj_chunk(2)                     # AR1(2)
                interleave(attn_steps(3), lnqkv_steps(4), n_attn(3), n_lnqkv(4))
                drain(mlp_steps(1))                  # RS2(1)
                ln2_chunk(2)
                outproj_chunk(3)                     # AR1(3)
                interleave(attn_steps(4), lnqkv_steps(5), n_attn(4), n_lnqkv(5))
                drain(mlp_steps(2))                  # RS2(2)
                ln2_chunk(3)
                outproj_chunk(4)                     # AR1(4)
                interleave(attn_steps(5), mlp_steps(3), n_attn(5), n_mlp(3))  # RS2(3)
                ln2_chunk(4)
                outproj_chunk(5)                     # AR1(5)
                drain(mlp_steps(4))                  # RS2(4)
                ln2_chunk(5)
                drain(mlp_steps(5))                  # RS2(5)

    _split_sync_waits(nc)
    return nc


@functools.lru_cache(maxsize=1)
def _get_nc():
    return _build_nc()


def _make_in_maps(inputs):
    x = np.asarray(inputs["x"], F32)
    W_qkv = np.asarray(inputs["W_qkv"], F32)
    b_qkv = np.asarray(inputs["b_qkv"], F32)
    W_o = np.asarray(inputs["W_o"], F32)
    b_o = np.asarray(inputs["b_o"], F32)
    ln1_g = np.asarray(inputs["ln1_g"], F32)
    ln1_b = np.asarray(inputs["ln1_b"], F32)
    ln2_g = np.asarray(inputs["ln2_g"], F32)
    ln2_b = np.asarray(inputs["ln2_b"], F32)
    W1 = np.asarray(inputs["W1"], F32)
    b1 = np.asarray(inputs["b1"], F32)
    W2 = np.asarray(inputs["W2"], F32)
    b2 = np.asarray(inputs["b2"], F32)

    scale = HS ** -0.5
    Wqkv_f = ln1_g[:, None] * W_qkv
    bqkv_f = ln1_b @ W_qkv + b_qkv
    Kw, Qw, Vw = Wqkv_f[:, :C], Wqkv_f[:, C:2 * C], Wqkv_f[:, 2 * C:]
    bK, bQ, bV = bqkv_f[:C], bqkv_f[C:2 * C], bqkv_f[2 * C:]
    W1f = ln2_g[:, None] * W1
    b1f = ln2_b @ W1 + b1

    ident = np.eye(128, dtype=BF16)
    mask = np.triu(np.ones((128, 128), dtype=F32)).astype(BF16)
    # b_o/TP folded into each rank's out-proj evacuation (AR sums to b_o)
    bob = np.ascontiguousarray(np.broadcast_to(b_o / TP, (128, C))).astype(F32)
    b2qc = np.ascontiguousarray(np.broadcast_to(b2, (128, C))).astype(F32)

    in_maps = []
    for core in range(NCORES):
        g, r = divmod(core, TP)
        hs = slice(CHR * r, CHR * (r + 1))
        hid = slice(HIDR * r, HIDR * (r + 1))
        xg = x[g]
        m = {
            "x_b": np.ascontiguousarray(xg),
            "wq": np.ascontiguousarray(Qw[:, hs] * scale).astype(BF16),
            "wk": np.ascontiguousarray(Kw[:, hs]).astype(BF16),
            "wv": np.ascontiguousarray(Vw[:, hs]).astype(BF16),
            "bq": np.ascontiguousarray((bQ[hs] * scale).reshape(2, 128).T),
            "bk": np.ascontiguousarray(bK[hs].reshape(2, 128).T),
            "bvb": np.ascontiguousarray(np.broadcast_to(bV[hs], (128, CHR))),
            "wo": np.ascontiguousarray(W_o[hs, :]).astype(BF16),
            "bob": bob,
            "w1": np.ascontiguousarray(W1f[:, hid]).astype(BF16),
            "b1": np.ascontiguousarray(b1f[hid].reshape(HIDR // 128, 128).T),
            "w2": np.ascontiguousarray(W2[hid, :]).astype(BF16),
            "bq4": b2qc,
            "ident": ident,
            "maskut": mask,
        }
        in_maps.append(m)
    return in_maps


def _run(inputs, trace=False):
    nc = _get_nc()
    in_maps = _make_in_maps(inputs)
    res = bass_utils.run_bass_kernel_spmd(
        nc, in_maps, core_ids=list(range(NCORES)), trace=trace
    )
    out = np.empty((B, T, C), F32)
    for core in range(NCORES):
        g, r = divmod(core, TP)
        o = np.asarray(res.results[core]["out"], dtype=F32)
        for (start, rows) in CHUNKS:
            q = rows // 4
            lo = start // 4
            out[g, start + r * q: start + (r + 1) * q] = o[lo:lo + q]
    return out, res


def kernel(**inputs) -> np.ndarray:
    out, _ = _run(inputs, trace=False)
    return out
